# revision 42
# baseline (speedup 1.0000x reference)
"""AttentionalGCN forward on 8 Trainium2 NeuronCores — fp8 A-stream, v3.

Math note: the reference's attention block is an exact no-op —
``einsum('ij,ik->ik', softmax(scores), agg) == rowsum(softmax) * agg == agg``
— so the output reduces to

    out = x @ (W_obj + W_skip) + r @ W_rel + A.T @ (x @ W_nobj) +
          colsum(A) x b_nobj + (b_obj + b_rel + b_skip)

Everything except the huge A.T @ P contraction (A is 8192x8192) is tiny
and is precomputed on the host:
  - P = x @ W_nobj in f32, cast e4m3 (single hi stream; measured rel err
    8.890e-3 vs the 2e-2 harness gate on the fixed seed-0 inputs,
    deterministic — the previous version's extra lo stream bought
    5.6e-4 for 0.5 MB, a net time loss),
  - proj = x @ (W_obj+W_skip) + r @ W_rel + biases + colsum(A) x b_nobj
    as [64, 1024] fp16 per core,
  - A cast to fp8 e4m3 (EXACT for a 0/1 matrix) and pre-tiled
    COLUMN-MAJOR: all 64 k-tiles of output columns 0-511 first, then
    512-1023, so half 0's combine + output DMA overlap half 1's stream.

Sharding: core m owns columns [m*1024, (m+1)*1024) of A (= rows of the
output); the host concatenates the 8 output shards.

Device program:
  - sync ring (HWDGE #1): the 8.4 MB A stream in even-k-tile chunks,
    small head chunks (the scalar ring keeps the SDMA engines fed while
    sync issues the next), one semaphore PER chunk (a cumulative
    counter races — see the comment at the semaphore declarations).
  - scalar ring (HWDGE #2): phl pieces + proj early, then the two
    output-half DMAs gated on the combines; nothing waits on dma_o (the
    NEFF epilogue's dma_reset drains the queues; a DMA with NO update at
    all SIGABRTs walrus codegen, so the inc stays).
  - tensor: warm-up junk matmuls on garbage SBUF, then one DoubleRow
    fp8 matmul per (k-tile pair, column half) into PSUM [64, 512]
    banks; stop + pe_h inc at the end of each half.
  - vector: one add per half (PSUM f32 + proj f16 -> out f16).

Measured ~38.0-40.5 us (median ~38.4-39.4) vs 43.1 us for the previous
row-major/hi+lo version in normal device windows; under the chip's P0
power-state downclock (PE ~494 ns/matmul instead of 379, DVE +20%) the
same binary measures ~40.5-43.4 — environmental, batch-dependent. Breakdown of a 38.3 us run (timestamps within
the measured window, which spans [first framework const memset ..
final framework barrier]): 1.2 us fixed entry; first A byte 8.6; first
real matmul 10.0; PE is the binding resource mid-stream and finishes
~34.8; combine h1 + output issue end ~36.1; then ~8 us fixed exit (the
NEFF wrapper resets all 253 kernel semaphores one EVENT_SEMAPHORE at a
time — walrus-emitted, not kernel-controllable).

TRN2 facts this is built around (measured on this part):
  - DR fp8 matmul pace = 512-cycle MM + SERIALIZED LDWEIGHTS (DoubleRow
    disables FWL at any FD; LDW time scales with stationary bytes):
    [128,2,64]w x [128,2,512]m = 379 ns warm / ~634 cold per instr, and
    a [128,2,128]w stationary measures 454 — so hi-only FD=64 beats
    both hi|lo and a duplicated-feature FD=128 layout. A stride-0
    broadcast AP does NOT re-enable FWL. 64 instrs x 379 = 24.3 us PE
    busy is the kernel's binding resource (the 8.9 MB stream takes
    ~24.7 us incl. the ~3 us sub-peak ramp of the first ~8 us).
  - plain (non-DR) fp8 runs at bf16 speed (1 elem/cell/cycle): 128
    instrs x ~216 = 27.6 us — worse. uint8 (DoublePixel) matmuls are
    rejected by bass (float dtypes only). DoubleRowSwInterleave fails
    the s3_lw_valid_num_active_cols HW check for this shape.
  - one HWDGE ring sustains ~423 GB/s mid-stream HBM->SBUF; the 16
    per-DMA-engine completion incs spread ~0.5-1.1 us after the data
    lands (engines 7/15 lag). dma_start issue costs ~0.6-0.7 us on the
    issuing engine. Both HWDGE rings (sync + scalar) share the ~435
    GB/s SBUF-AXI fabric cap, so the second ring buys issue/ordering
    parallelism, not bandwidth.
  - the HAM clock gate needs ~3.4 us of SUSTAINED PE busy; idle gaps
    >3.4 us re-throttle to K=4/8 (half clock). GAP_JUNK fills the head
    receipt gaps.
  - the 379 ns decomposes as clock-base (213 @2.4 GHz) + ~120-165 ns of
    SBUF contention with the concurrent A-stream: an isolated probe of
    the SAME matmul shape with no DMA running paces at 259 ns in a
    P0-throttled window (256-cycle clock base, accumulation chains and
    weight reuse both free — 80 LDWEIGHTS for 80 matmuls, no dedup).
    Stream-in (423 GB/s) + PE moving-reads (346 GB/s) sit at a shared
    ~770 GB/s SBUF roofline: A's bytes cross SBUF twice by
    construction, so stream and PE are CO-PACED — speeding either side
    alone just starves the other (measured as a wash repeatedly).
  - back-to-back matmuls must keep the same base partitions -
    alternating tile_position crashes the device (NRT 101).
  - a kernel-internal wait must cover a DMA semaphore's full
    accumulated total (the epilogue sem-clear handles the rest).
"""

from contextlib import ExitStack

import numpy as np
import ml_dtypes

import concourse.bass as bass
import concourse.bacc as bacc
from concourse import mybir
from concourse import bass_utils

N = 8192          # nodes
D = 64            # feature dim
M = 8             # cores
SH = N // M       # 1024 output rows / A columns per core
KT = N // 128     # 64 contraction k-tiles of 128 rows
HW = SH // 2      # 512 columns per half
F8 = mybir.dt.float8e4
F16 = mybir.dt.float16
F32 = mybir.dt.float32

# A-stream chunks in k-tiles, per column half (all even so DoubleRow
# pairs never straddle a chunk boundary). Small head chunks — the
# scalar ring keeps the SDMA engines busy while sync issues the next —
# and a tiny last chunk to cut the final receipt latency.
CHUNKS_H0 = [2, 4, 8, 16, 16, 18]
CHUNKS_H1 = [16, 16, 16, 12, 2, 2]
assert sum(CHUNKS_H0) == KT and sum(CHUNKS_H1) == KT
CHUNKS = [(0, c) for c in CHUNKS_H0] + [(1, c) for c in CHUNKS_H1]
# phl pieces in k-tiles (even); piece 0 small so the first matmul can
# fire as soon as piece 0 + A chunk 0 land.
PHL_PC = [8, 24, 32]
PHL_CS = [sum(PHL_PC[:i]) for i in range(len(PHL_PC))]
N_JUNK = 6        # PE warm-up matmuls on garbage SBUF
# short (256-col) junk matmuls inserted after the first h0 chunks'
# real matmuls: the head receipts arrive ~2.5-2.9 us apart while the
# real work per head chunk is <=1.2 us, and the HAM clock gate needs
# ~3.4 us of SUSTAINED busy — idle gaps re-cool it (measured 379-634 ns
# per real matmul without this vs 216 warm).
GAP_JUNK = {0: 4, 1: 4, 2: 3, 3: 2}

_BUILT = {}


def build_bass():
    """One SPMD program, identical on all 8 cores; per-core data differs."""
    nc = bacc.Bacc("TRN2", target_bir_lowering=False, debug=False, num_devices=M)

    # phl[p, k*64+f] = P_hi[k*128+p, f]
    phl = nc.declare_dram_parameter("phl", [128, KT * D], F8, isOutput=False)
    proj = nc.declare_dram_parameter("proj", [D, SH], F16, isOutput=False)
    # column-major pre-tiled fp8: a8[p, ((h*KT)+k)*512 + j] =
    #   A[k*128 + p, m*1024 + h*512 + j] for this core's block m
    a8 = nc.declare_dram_parameter("a8", [128, 2 * KT * HW], F8, isOutput=False)
    outT = nc.declare_dram_parameter("outT", [D, SH], F16, isOutput=True)

    with ExitStack() as ctx:
        phl_sb = ctx.enter_context(nc.sbuf_tensor("phl_sb", [128, KT, D], F8))
        proj_sb = ctx.enter_context(nc.sbuf_tensor("proj_sb", [D, SH], F16))
        a8_sb = ctx.enter_context(nc.sbuf_tensor("a8_sb", [128, 2, KT, HW], F8))
        junk = ctx.enter_context(nc.sbuf_tensor("junk", [128, 640], F8))
        out_sb = ctx.enter_context(nc.sbuf_tensor("out_sb", [D, SH], F16))
        po2 = ctx.enter_context(nc.psum_tensor("po2", [D, SH], F32))
        scr = ctx.enter_context(nc.psum_tensor("scr", [128, 512], F32))

        # one semaphore PER chunk/piece: a single cumulative counter is
        # UNSOUND — the 16 SDMA engines complete their slices of queued
        # DMAs independently, so fast engines running ahead on chunk c+1
        # can raise the total to 16*(c+1) while a lagging engine (7/15
        # lag ~1 us) hasn't landed chunk c; the matmul then reads
        # unlanded SBUF (fp8 0xFF garbage = NaN). Observed ~1/6 runs.
        a_sems = [ctx.enter_context(nc.semaphore(f"a_sem{c}"))
                  for c in range(len(CHUNKS))]
        phl_sems = [ctx.enter_context(nc.semaphore(f"phl_sem{i}"))
                    for i in range(len(PHL_PC))]
        proj_sem = ctx.enter_context(nc.semaphore("proj_sem"))
        pe_h = [ctx.enter_context(nc.semaphore(f"pe_h{h}")) for h in range(2)]
        dve_o = [ctx.enter_context(nc.semaphore(f"dve_o{h}")) for h in range(2)]
        dma_o = ctx.enter_context(nc.semaphore("dma_o"))
        block = ctx.enter_context(nc.Block(no_gpsimd_drain=True))

        @block.sync
        def _(sync):
            # the whole A stream, in consumption order, on HWDGE ring 1
            off = {0: 0, 1: KT * HW}
            cs = {0: 0, 1: 0}
            for ci, (h, w) in enumerate(CHUNKS):
                c0 = cs[h]
                sync.dma_start(
                    a8_sb[:, h, c0:c0 + w, :],
                    a8[:, off[h] + c0 * HW:off[h] + (c0 + w) * HW],
                ).then_inc(a_sems[ci], 16)
                cs[h] = c0 + w
            # no final wait: the NEFF epilogue drains the DMA queues.

        @block.scalar
        def _(scalar):
            # phl + proj early on HWDGE ring 2 (concurrent with ring 1)
            for i, w in enumerate(PHL_PC):
                scalar.dma_start(
                    phl_sb[:, PHL_CS[i]:PHL_CS[i] + w, :],
                    phl[:, PHL_CS[i] * D:(PHL_CS[i] + w) * D],
                ).then_inc(phl_sems[i], 16)
            scalar.dma_start(proj_sb[:], proj[:]).then_inc(proj_sem, 16)
            # output halves as soon as each combine lands; walrus codegen
            # requires a completion inc on every DMA, but nothing WAITS on
            # dma_o — the NEFF epilogue's dma_reset drains the queue.
            for h in range(2):
                hsl = slice(h * HW, (h + 1) * HW)
                scalar.wait_ge(dve_o[h], 1)
                scalar.dma_start(outT[:, hsl], out_sb[:, hsl]).then_inc(
                    dma_o, 16)

        @block.tensor
        def _(tensor):
            # HAM warm-up on garbage SBUF (never read; fp8 NaNs are inert)
            for _ in range(N_JUNK):
                tensor.matmul(scr[:, 0:512], junk[:, 0:128], junk[:, 128:640],
                              start=True, stop=True)
            chunks_done = 0
            qt = 0
            for h in range(2):
                hsl = slice(h * HW, (h + 1) * HW)
                covered = -1
                for ci, w in enumerate(CHUNKS_H0 if h == 0 else CHUNKS_H1):
                    tensor.wait_ge(a_sems[chunks_done], 16)
                    chunks_done += 1
                    c0 = sum((CHUNKS_H0 if h == 0 else CHUNKS_H1)[:ci])
                    if h == 0:
                        while covered < c0 + w - 1:
                            tensor.wait_ge(phl_sems[qt], 16)
                            covered = PHL_CS[qt] + PHL_PC[qt] - 1
                            qt += 1
                    for t in range(w // 2):
                        k = c0 + 2 * t
                        last = k == KT - 2
                        mm = tensor.matmul(
                            po2[:, hsl],
                            phl_sb[:, k:k + 2, :],
                            a8_sb[:, h, k:k + 2, :],
                            start=k == 0,
                            stop=last,
                            perf_mode=mybir.MatmulPerfMode.DoubleRow,
                        )
                        if last:
                            mm.then_inc(pe_h[h], 1)
                    if h == 0:
                        for _ in range(GAP_JUNK.get(ci, 0)):
                            tensor.matmul(scr[:, 0:256], junk[:, 0:128],
                                          junk[:, 128:384],
                                          start=True, stop=True)

        @block.vector
        def _(vector):
            vector.wait_ge(proj_sem, 16)
            for h in range(2):
                hsl = slice(h * HW, (h + 1) * HW)
                vector.wait_ge(pe_h[h], 1)
                vector.tensor_add(
                    out_sb[:, hsl], po2[:, hsl], proj_sb[:, hsl]
                ).then_inc(dve_o[h], 1)

    nc.compile()
    return nc


def _prep_in_maps(object_features, relationship_features, adjacency_matrix,
                  W_obj, b_obj, W_nobj, b_nobj, W_rel, b_rel,
                  W_skip, b_skip):
    x = np.ascontiguousarray(object_features, dtype=np.float32)
    r = np.ascontiguousarray(relationship_features, dtype=np.float32)
    A = np.asarray(adjacency_matrix, dtype=np.float32)

    # P = x @ W_nobj, e4m3, k-tiled: phl[p, k*64+f] = P[k*128+p, f]
    P = x @ np.asarray(W_nobj, dtype=np.float32)                 # [N, D]
    phi = P.astype(ml_dtypes.float8_e4m3)
    phl = np.ascontiguousarray(
        phi.reshape(KT, 128, D).transpose(1, 0, 2).reshape(128, KT * D))

    # proj = x @ (W_obj+W_skip) + r @ W_rel + biases + colsum(A) x b_nobj
    colsum = A.sum(axis=0, dtype=np.float32)                     # [N]
    proj_full = (
        x @ (np.asarray(W_obj) + np.asarray(W_skip))
        + r @ np.asarray(W_rel)
        + (np.asarray(b_obj) + np.asarray(b_rel) + np.asarray(b_skip))[None, :]
        + colsum[:, None] * np.asarray(b_nobj)[None, :]
    ).T.astype(np.float16)                                       # [D, N]

    in_maps = []
    for m in range(M):
        sl = slice(m * SH, (m + 1) * SH)
        # column-major pre-tile: [k, p, h, j] -> [p, h, k, j]; exact fp8
        blk = A[:, sl].astype(ml_dtypes.float8_e4m3)             # [8192, 1024]
        blk = np.ascontiguousarray(
            blk.reshape(KT, 128, 2, HW).transpose(1, 2, 0, 3)
            .reshape(128, 2 * KT * HW))
        in_maps.append({
            "phl": phl,
            "proj": np.ascontiguousarray(proj_full[:, sl]),
            "a8": blk,
        })
    return in_maps


def run(inputs: dict, **run_kwargs):
    """Build (cached), run on cores 0-7, return (output, BassKernelResults)."""
    if "nc" not in _BUILT:
        _BUILT["nc"] = build_bass()
    nc = _BUILT["nc"]
    in_maps = _prep_in_maps(
        inputs["object_features"], inputs["relationship_features"],
        inputs["adjacency_matrix"],
        inputs["W_obj"], inputs["b_obj"], inputs["W_nobj"], inputs["b_nobj"],
        inputs["W_rel"], inputs["b_rel"], inputs["W_skip"], inputs["b_skip"],
    )
    last_err = None
    for attempt in range(3):
        try:
            res = bass_utils.run_bass_kernel_spmd(
                nc, in_maps, core_ids=list(range(M)), **run_kwargs
            )
            break
        except Exception as e:  # transient NRT device errors do occur
            last_err = e
            if attempt == 2:
                raise
            import time
            time.sleep(2.0)
    out = np.concatenate(
        [res.results[m]["outT"].T for m in range(M)], axis=0
    ).astype(np.float32)
    return out, res


def kernel(**inputs) -> np.ndarray:
    out, _ = run(inputs)
    return out


# revision 44
# speedup vs baseline: 1.0561x; 1.0561x over previous
"""AttentionalGCN forward on 8 Trainium2 NeuronCores — fp8 A-stream, v3.

Math note: the reference's attention block is an exact no-op —
``einsum('ij,ik->ik', softmax(scores), agg) == rowsum(softmax) * agg == agg``
— so the output reduces to

    out = x @ (W_obj + W_skip) + r @ W_rel + A.T @ (x @ W_nobj) +
          colsum(A) x b_nobj + (b_obj + b_rel + b_skip)

Everything except the huge A.T @ P contraction (A is 8192x8192) is tiny
and is precomputed on the host:
  - P = x @ W_nobj in f32, cast e4m3 (single hi stream; measured rel err
    8.890e-3 vs the 2e-2 harness gate on the fixed seed-0 inputs,
    deterministic — the previous version's extra lo stream bought
    5.6e-4 for 0.5 MB, a net time loss),
  - proj = x @ (W_obj+W_skip) + r @ W_rel + biases + colsum(A) x b_nobj
    as [64, 1024] fp16 per core,
  - A cast to fp8 e4m3 (EXACT for a 0/1 matrix) and pre-tiled
    COLUMN-MAJOR: all 64 k-tiles of output columns 0-511 first, then
    512-1023, so half 0's combine + output DMA overlap half 1's stream.

Sharding: core m owns columns [m*1024, (m+1)*1024) of A (= rows of the
output); the host concatenates the 8 output shards.

Device program:
  - sync ring (HWDGE #1): the 8.4 MB A stream in even-k-tile chunks,
    small head chunks (the scalar ring keeps the SDMA engines fed while
    sync issues the next), one semaphore PER chunk (a cumulative
    counter races — see the comment at the semaphore declarations).
  - scalar ring (HWDGE #2): phl pieces + proj early, then the two
    output-half DMAs gated on the combines; nothing waits on dma_o (the
    NEFF epilogue's dma_reset drains the queues; a DMA with NO update at
    all SIGABRTs walrus codegen, so the inc stays).
  - tensor: warm-up junk matmuls on garbage SBUF, then one DoubleRow
    fp8 matmul per (k-tile pair, column half) into PSUM [64, 512]
    banks; stop + pe_h inc at the end of each half.
  - vector: one add per half (PSUM f32 + proj f16 -> out f16).

Measured ~38.0-40.5 us (median ~38.4-39.4) vs 43.1 us for the previous
row-major/hi+lo version in normal device windows; under the chip's P0
power-state downclock (PE ~494 ns/matmul instead of 379, DVE +20%) the
same binary measures ~40.5-43.4 — environmental, batch-dependent. Breakdown of a 38.3 us run (timestamps within
the measured window, which spans [first framework const memset ..
final framework barrier]): 1.2 us fixed entry; first A byte 8.6; first
real matmul 10.0; PE is the binding resource mid-stream and finishes
~34.8; combine h1 + output issue end ~36.1; then ~8 us fixed exit (the
NEFF wrapper resets all 253 kernel semaphores one EVENT_SEMAPHORE at a
time — walrus-emitted, not kernel-controllable).

TRN2 facts this is built around (measured on this part):
  - DR fp8 matmul pace = 512-cycle MM + SERIALIZED LDWEIGHTS (DoubleRow
    disables FWL at any FD; LDW time scales with stationary bytes):
    [128,2,64]w x [128,2,512]m = 379 ns warm / ~634 cold per instr, and
    a [128,2,128]w stationary measures 454 — so hi-only FD=64 beats
    both hi|lo and a duplicated-feature FD=128 layout. A stride-0
    broadcast AP does NOT re-enable FWL. 64 instrs x 379 = 24.3 us PE
    busy is the kernel's binding resource (the 8.9 MB stream takes
    ~24.7 us incl. the ~3 us sub-peak ramp of the first ~8 us).
  - plain (non-DR) fp8 runs at bf16 speed (1 elem/cell/cycle): 128
    instrs x ~216 = 27.6 us — worse. uint8 (DoublePixel) matmuls are
    rejected by bass (float dtypes only). DoubleRowSwInterleave fails
    the s3_lw_valid_num_active_cols HW check for this shape.
  - one HWDGE ring sustains ~423 GB/s mid-stream HBM->SBUF; the 16
    per-DMA-engine completion incs spread ~0.5-1.1 us after the data
    lands (engines 7/15 lag). dma_start issue costs ~0.6-0.7 us on the
    issuing engine. Both HWDGE rings (sync + scalar) share the ~435
    GB/s SBUF-AXI fabric cap, so the second ring buys issue/ordering
    parallelism, not bandwidth.
  - the HAM clock gate needs ~3.4 us of SUSTAINED PE busy; idle gaps
    >3.4 us re-throttle to K=4/8 (half clock). GAP_JUNK fills the head
    receipt gaps.
  - the 379 ns decomposes as clock-base (213 @2.4 GHz) + ~120-165 ns of
    SBUF contention with the concurrent A-stream: an isolated probe of
    the SAME matmul shape with no DMA running paces at 259 ns in a
    P0-throttled window (256-cycle clock base, accumulation chains and
    weight reuse both free — 80 LDWEIGHTS for 80 matmuls, no dedup).
    Stream-in (423 GB/s) + PE moving-reads (346 GB/s) sit at a shared
    ~770 GB/s SBUF roofline: A's bytes cross SBUF twice by
    construction, so stream and PE are CO-PACED — speeding either side
    alone just starves the other (measured as a wash repeatedly).
  - back-to-back matmuls must keep the same base partitions -
    alternating tile_position crashes the device (NRT 101).
  - a kernel-internal wait must cover a DMA semaphore's full
    accumulated total (the epilogue sem-clear handles the rest).
"""

from contextlib import ExitStack

import numpy as np
import ml_dtypes

import concourse.bass as bass
import concourse.bacc as bacc
from concourse import mybir
from concourse import bass_utils

N = 8192          # nodes
D = 64            # feature dim
M = 8             # cores
SH = N // M       # 1024 output rows / A columns per core
KT = N // 128     # 64 contraction k-tiles of 128 rows
HW = SH // 2      # 512 columns per half
F8 = mybir.dt.float8e4
F16 = mybir.dt.float16
F32 = mybir.dt.float32

# A-stream chunks in k-tiles, per column half (all even so DoubleRow
# pairs never straddle a chunk boundary). Small head chunks — the
# scalar ring keeps the SDMA engines busy while sync issues the next —
# and a tiny last chunk to cut the final receipt latency.
CHUNKS_H0 = [2, 4, 8, 16, 16, 18]
CHUNKS_H1 = [16, 16, 16, 12, 2, 2]
assert sum(CHUNKS_H0) == KT and sum(CHUNKS_H1) == KT
CHUNKS = [(0, c) for c in CHUNKS_H0] + [(1, c) for c in CHUNKS_H1]
# phl pieces in k-tiles (even); piece 0 small so the first matmul can
# fire as soon as piece 0 + A chunk 0 land.
PHL_PC = [8, 24, 32]
PHL_CS = [sum(PHL_PC[:i]) for i in range(len(PHL_PC))]
N_JUNK = 6        # PE warm-up matmuls on garbage SBUF
# extra short junk matmuls after the first h0 chunks (HAM warm-keeping)
# measured as a wash in an interleaved A/B under the SBUF-contention
# regime (gap-ON mean 40.75 us vs gap-OFF 40.13, n=10 each) — keep off.
GAP_JUNK = {}

_BUILT = {}


def build_bass():
    """One SPMD program, identical on all 8 cores; per-core data differs."""
    nc = bacc.Bacc("TRN2", target_bir_lowering=False, debug=False, num_devices=M)

    # phl[p, k*64+f] = P_hi[k*128+p, f]
    phl = nc.declare_dram_parameter("phl", [128, KT * D], F8, isOutput=False)
    proj = nc.declare_dram_parameter("proj", [D, SH], F16, isOutput=False)
    # column-major pre-tiled fp8: a8[p, ((h*KT)+k)*512 + j] =
    #   A[k*128 + p, m*1024 + h*512 + j] for this core's block m
    a8 = nc.declare_dram_parameter("a8", [128, 2 * KT * HW], F8, isOutput=False)
    outT = nc.declare_dram_parameter("outT", [D, SH], F16, isOutput=True)

    with ExitStack() as ctx:
        phl_sb = ctx.enter_context(nc.sbuf_tensor("phl_sb", [128, KT, D], F8))
        proj_sb = ctx.enter_context(nc.sbuf_tensor("proj_sb", [D, SH], F16))
        a8_sb = ctx.enter_context(nc.sbuf_tensor("a8_sb", [128, 2, KT, HW], F8))
        junk = ctx.enter_context(nc.sbuf_tensor("junk", [128, 640], F8))
        out_sb = ctx.enter_context(nc.sbuf_tensor("out_sb", [D, SH], F16))
        po2 = ctx.enter_context(nc.psum_tensor("po2", [D, SH], F32))
        scr = ctx.enter_context(nc.psum_tensor("scr", [128, 512], F32))

        # one semaphore PER chunk/piece: a single cumulative counter is
        # UNSOUND — the 16 SDMA engines complete their slices of queued
        # DMAs independently, so fast engines running ahead on chunk c+1
        # can raise the total to 16*(c+1) while a lagging engine (7/15
        # lag ~1 us) hasn't landed chunk c; the matmul then reads
        # unlanded SBUF (fp8 0xFF garbage = NaN). Observed ~1/6 runs.
        a_sems = [ctx.enter_context(nc.semaphore(f"a_sem{c}"))
                  for c in range(len(CHUNKS))]
        phl_sems = [ctx.enter_context(nc.semaphore(f"phl_sem{i}"))
                    for i in range(len(PHL_PC))]
        proj_sem = ctx.enter_context(nc.semaphore("proj_sem"))
        pe_h = [ctx.enter_context(nc.semaphore(f"pe_h{h}")) for h in range(2)]
        dve_o = [ctx.enter_context(nc.semaphore(f"dve_o{h}")) for h in range(2)]
        dma_o = ctx.enter_context(nc.semaphore("dma_o"))
        block = ctx.enter_context(nc.Block(no_gpsimd_drain=True))

        @block.sync
        def _(sync):
            # the whole A stream, in consumption order, on HWDGE ring 1
            off = {0: 0, 1: KT * HW}
            cs = {0: 0, 1: 0}
            for ci, (h, w) in enumerate(CHUNKS):
                c0 = cs[h]
                sync.dma_start(
                    a8_sb[:, h, c0:c0 + w, :],
                    a8[:, off[h] + c0 * HW:off[h] + (c0 + w) * HW],
                ).then_inc(a_sems[ci], 16)
                cs[h] = c0 + w
            # no final wait: the NEFF epilogue drains the DMA queues.

        @block.scalar
        def _(scalar):
            # phl + proj early on HWDGE ring 2 (concurrent with ring 1)
            for i, w in enumerate(PHL_PC):
                scalar.dma_start(
                    phl_sb[:, PHL_CS[i]:PHL_CS[i] + w, :],
                    phl[:, PHL_CS[i] * D:(PHL_CS[i] + w) * D],
                ).then_inc(phl_sems[i], 16)
            scalar.dma_start(proj_sb[:], proj[:]).then_inc(proj_sem, 16)
            # output halves as soon as each combine lands; walrus codegen
            # requires a completion inc on every DMA, but nothing WAITS on
            # dma_o — the NEFF epilogue's dma_reset drains the queue.
            for h in range(2):
                hsl = slice(h * HW, (h + 1) * HW)
                scalar.wait_ge(dve_o[h], 1)
                scalar.dma_start(outT[:, hsl], out_sb[:, hsl]).then_inc(
                    dma_o, 16)

        @block.tensor
        def _(tensor):
            # HAM warm-up on garbage SBUF (never read; fp8 NaNs are inert)
            for _ in range(N_JUNK):
                tensor.matmul(scr[:, 0:512], junk[:, 0:128], junk[:, 128:640],
                              start=True, stop=True)
            chunks_done = 0
            qt = 0
            for h in range(2):
                hsl = slice(h * HW, (h + 1) * HW)
                covered = -1
                for ci, w in enumerate(CHUNKS_H0 if h == 0 else CHUNKS_H1):
                    tensor.wait_ge(a_sems[chunks_done], 16)
                    chunks_done += 1
                    c0 = sum((CHUNKS_H0 if h == 0 else CHUNKS_H1)[:ci])
                    if h == 0:
                        while covered < c0 + w - 1:
                            tensor.wait_ge(phl_sems[qt], 16)
                            covered = PHL_CS[qt] + PHL_PC[qt] - 1
                            qt += 1
                    for t in range(w // 2):
                        k = c0 + 2 * t
                        last = k == KT - 2
                        mm = tensor.matmul(
                            po2[:, hsl],
                            phl_sb[:, k:k + 2, :],
                            a8_sb[:, h, k:k + 2, :],
                            start=k == 0,
                            stop=last,
                            perf_mode=mybir.MatmulPerfMode.DoubleRow,
                        )
                        if last:
                            mm.then_inc(pe_h[h], 1)
                    if h == 0:
                        for _ in range(GAP_JUNK.get(ci, 0)):
                            tensor.matmul(scr[:, 0:256], junk[:, 0:128],
                                          junk[:, 128:384],
                                          start=True, stop=True)

        @block.vector
        def _(vector):
            vector.wait_ge(proj_sem, 16)
            for h in range(2):
                hsl = slice(h * HW, (h + 1) * HW)
                vector.wait_ge(pe_h[h], 1)
                vector.tensor_add(
                    out_sb[:, hsl], po2[:, hsl], proj_sb[:, hsl]
                ).then_inc(dve_o[h], 1)

    nc.compile()
    return nc


def _prep_in_maps(object_features, relationship_features, adjacency_matrix,
                  W_obj, b_obj, W_nobj, b_nobj, W_rel, b_rel,
                  W_skip, b_skip):
    x = np.ascontiguousarray(object_features, dtype=np.float32)
    r = np.ascontiguousarray(relationship_features, dtype=np.float32)
    A = np.asarray(adjacency_matrix, dtype=np.float32)

    # P = x @ W_nobj, e4m3, k-tiled: phl[p, k*64+f] = P[k*128+p, f]
    P = x @ np.asarray(W_nobj, dtype=np.float32)                 # [N, D]
    phi = P.astype(ml_dtypes.float8_e4m3)
    phl = np.ascontiguousarray(
        phi.reshape(KT, 128, D).transpose(1, 0, 2).reshape(128, KT * D))

    # proj = x @ (W_obj+W_skip) + r @ W_rel + biases + colsum(A) x b_nobj
    colsum = A.sum(axis=0, dtype=np.float32)                     # [N]
    proj_full = (
        x @ (np.asarray(W_obj) + np.asarray(W_skip))
        + r @ np.asarray(W_rel)
        + (np.asarray(b_obj) + np.asarray(b_rel) + np.asarray(b_skip))[None, :]
        + colsum[:, None] * np.asarray(b_nobj)[None, :]
    ).T.astype(np.float16)                                       # [D, N]

    in_maps = []
    for m in range(M):
        sl = slice(m * SH, (m + 1) * SH)
        # column-major pre-tile: [k, p, h, j] -> [p, h, k, j]; exact fp8
        blk = A[:, sl].astype(ml_dtypes.float8_e4m3)             # [8192, 1024]
        blk = np.ascontiguousarray(
            blk.reshape(KT, 128, 2, HW).transpose(1, 2, 0, 3)
            .reshape(128, 2 * KT * HW))
        in_maps.append({
            "phl": phl,
            "proj": np.ascontiguousarray(proj_full[:, sl]),
            "a8": blk,
        })
    return in_maps


def run(inputs: dict, **run_kwargs):
    """Build (cached), run on cores 0-7, return (output, BassKernelResults)."""
    if "nc" not in _BUILT:
        _BUILT["nc"] = build_bass()
    nc = _BUILT["nc"]
    in_maps = _prep_in_maps(
        inputs["object_features"], inputs["relationship_features"],
        inputs["adjacency_matrix"],
        inputs["W_obj"], inputs["b_obj"], inputs["W_nobj"], inputs["b_nobj"],
        inputs["W_rel"], inputs["b_rel"], inputs["W_skip"], inputs["b_skip"],
    )
    last_err = None
    for attempt in range(3):
        try:
            res = bass_utils.run_bass_kernel_spmd(
                nc, in_maps, core_ids=list(range(M)), **run_kwargs
            )
            break
        except Exception as e:  # transient NRT device errors do occur
            last_err = e
            if attempt == 2:
                raise
            import time
            time.sleep(2.0)
    out = np.concatenate(
        [res.results[m]["outT"].T for m in range(M)], axis=0
    ).astype(np.float32)
    return out, res


def kernel(**inputs) -> np.ndarray:
    out, _ = run(inputs)
    return out


# revision 46
# speedup vs baseline: 1.0676x; 1.0110x over previous
"""AttentionalGCN forward on 8 Trainium2 NeuronCores — fp8 A-stream, v3.

Math note: the reference's attention block is an exact no-op —
``einsum('ij,ik->ik', softmax(scores), agg) == rowsum(softmax) * agg == agg``
— so the output reduces to

    out = x @ (W_obj + W_skip) + r @ W_rel + A.T @ (x @ W_nobj) +
          colsum(A) x b_nobj + (b_obj + b_rel + b_skip)

Everything except the huge A.T @ P contraction (A is 8192x8192) is tiny
and is precomputed on the host:
  - P = x @ W_nobj in f32, cast e4m3 (single hi stream; measured rel err
    8.890e-3 vs the 2e-2 harness gate on the fixed seed-0 inputs,
    deterministic — the previous version's extra lo stream bought
    5.6e-4 for 0.5 MB, a net time loss),
  - proj = x @ (W_obj+W_skip) + r @ W_rel + biases + colsum(A) x b_nobj
    as [64, 1024] fp16 per core,
  - A cast to fp8 e4m3 (EXACT for a 0/1 matrix) and pre-tiled
    COLUMN-MAJOR: all 64 k-tiles of output columns 0-511 first, then
    512-1023, so half 0's combine + output DMA overlap half 1's stream.

Sharding: core m owns columns [m*1024, (m+1)*1024) of A (= rows of the
output); the host concatenates the 8 output shards.

Device program:
  - sync ring (HWDGE #1): the 8.4 MB A stream in even-k-tile chunks,
    small head chunks (the scalar ring keeps the SDMA engines fed while
    sync issues the next), one semaphore PER chunk (a cumulative
    counter races — see the comment at the semaphore declarations).
  - scalar ring (HWDGE #2): phl pieces + proj early, then the two
    output-half DMAs gated on the combines; nothing waits on dma_o (the
    NEFF epilogue's dma_reset drains the queues; a DMA with NO update at
    all SIGABRTs walrus codegen, so the inc stays).
  - tensor: warm-up junk matmuls on garbage SBUF, then one DoubleRow
    fp8 matmul per (k-tile pair, column half) into PSUM [64, 512]
    banks; stop + pe_h inc at the end of each half.
  - vector: one add per half (PSUM f32 + proj f16 -> out f16).

Measured ~38.0-40.5 us (median ~38.4-39.4) vs 43.1 us for the previous
row-major/hi+lo version in normal device windows; under the chip's P0
power-state downclock (PE ~494 ns/matmul instead of 379, DVE +20%) the
same binary measures ~40.5-43.4 — environmental, batch-dependent. Breakdown of a 38.3 us run (timestamps within
the measured window, which spans [first framework const memset ..
final framework barrier]): 1.2 us fixed entry; first A byte 8.6; first
real matmul 10.0; PE is the binding resource mid-stream and finishes
~34.8; combine h1 + output issue end ~36.1; then ~8 us fixed exit (the
NEFF wrapper resets all 253 kernel semaphores one EVENT_SEMAPHORE at a
time — walrus-emitted, not kernel-controllable).

TRN2 facts this is built around (measured on this part):
  - DR fp8 matmul pace = 512-cycle MM + SERIALIZED LDWEIGHTS (DoubleRow
    disables FWL at any FD; LDW time scales with stationary bytes):
    [128,2,64]w x [128,2,512]m = 379 ns warm / ~634 cold per instr, and
    a [128,2,128]w stationary measures 454 — so hi-only FD=64 beats
    both hi|lo and a duplicated-feature FD=128 layout. A stride-0
    broadcast AP does NOT re-enable FWL. 64 instrs x 379 = 24.3 us PE
    busy is the kernel's binding resource (the 8.9 MB stream takes
    ~24.7 us incl. the ~3 us sub-peak ramp of the first ~8 us).
  - plain (non-DR) fp8 runs at bf16 speed (1 elem/cell/cycle): 128
    instrs x ~216 = 27.6 us — worse. uint8 (DoublePixel) matmuls are
    rejected by bass (float dtypes only). DoubleRowSwInterleave fails
    the s3_lw_valid_num_active_cols HW check for this shape.
  - one HWDGE ring sustains ~423 GB/s mid-stream HBM->SBUF; the 16
    per-DMA-engine completion incs spread ~0.5-1.1 us after the data
    lands (engines 7/15 lag). dma_start issue costs ~0.6-0.7 us on the
    issuing engine. Both HWDGE rings (sync + scalar) share the ~435
    GB/s SBUF-AXI fabric cap, so the second ring buys issue/ordering
    parallelism, not bandwidth.
  - the HAM clock gate needs ~3.4 us of SUSTAINED PE busy; idle gaps
    >3.4 us re-throttle to K=4/8 (half clock). GAP_JUNK fills the head
    receipt gaps.
  - the 379 ns decomposes as clock-base (213 @2.4 GHz) + ~120-165 ns of
    SBUF contention with the concurrent A-stream: an isolated probe of
    the SAME matmul shape with no DMA running paces at 259 ns in a
    P0-throttled window (256-cycle clock base, accumulation chains and
    weight reuse both free — 80 LDWEIGHTS for 80 matmuls, no dedup).
    Stream-in (423 GB/s) + PE moving-reads (346 GB/s) sit at a shared
    ~770 GB/s SBUF roofline: A's bytes cross SBUF twice by
    construction, so stream and PE are CO-PACED — speeding either side
    alone just starves the other (measured as a wash repeatedly).
  - back-to-back matmuls must keep the same base partitions -
    alternating tile_position crashes the device (NRT 101).
  - a kernel-internal wait must cover a DMA semaphore's full
    accumulated total (the epilogue sem-clear handles the rest).
"""

from contextlib import ExitStack

import numpy as np
import ml_dtypes

import concourse.bass as bass
import concourse.bacc as bacc
from concourse import mybir
from concourse import bass_utils

N = 8192          # nodes
D = 64            # feature dim
M = 8             # cores
SH = N // M       # 1024 output rows / A columns per core
KT = N // 128     # 64 contraction k-tiles of 128 rows
HW = SH // 2      # 512 columns per half
F8 = mybir.dt.float8e4
F16 = mybir.dt.float16
F32 = mybir.dt.float32

# A-stream chunks in k-tiles, per column half (all even so DoubleRow
# pairs never straddle a chunk boundary). Small head chunks — the
# scalar ring keeps the SDMA engines busy while sync issues the next —
# and a tiny last chunk to cut the final receipt latency.
CHUNKS_H0 = [2, 4, 8, 16, 16, 18]
CHUNKS_H1 = [16, 16, 16, 12, 2, 2]
assert sum(CHUNKS_H0) == KT and sum(CHUNKS_H1) == KT
CHUNKS = [(0, c) for c in CHUNKS_H0] + [(1, c) for c in CHUNKS_H1]
# phl pieces in k-tiles (even); piece 0 small so the first matmul can
# fire as soon as piece 0 + A chunk 0 land.
PHL_PC = [8, 24, 32]
PHL_CS = [sum(PHL_PC[:i]) for i in range(len(PHL_PC))]
N_JUNK = 6        # PE warm-up matmuls on garbage SBUF
# extra short junk matmuls after the first h0 chunks (HAM warm-keeping)
# measured as a wash in an interleaved A/B under the SBUF-contention
# regime (gap-ON mean 40.75 us vs gap-OFF 40.13, n=10 each) — keep off.
GAP_JUNK = {}

_BUILT = {}


def build_bass():
    """One SPMD program, identical on all 8 cores; per-core data differs."""
    nc = bacc.Bacc("TRN2", target_bir_lowering=False, debug=False, num_devices=M)

    # phl[p, k*64+f] = P_hi[k*128+p, f]
    phl = nc.declare_dram_parameter("phl", [128, KT * D], F8, isOutput=False)
    proj = nc.declare_dram_parameter("proj", [D, SH], F16, isOutput=False)
    # column-major pre-tiled fp8: a8[p, ((h*KT)+k)*512 + j] =
    #   A[k*128 + p, m*1024 + h*512 + j] for this core's block m
    a8 = nc.declare_dram_parameter("a8", [128, 2 * KT * HW], F8, isOutput=False)
    outT = nc.declare_dram_parameter("outT", [D, SH], F16, isOutput=True)

    with ExitStack() as ctx:
        phl_sb = ctx.enter_context(nc.sbuf_tensor("phl_sb", [128, KT, D], F8))
        proj_sb = ctx.enter_context(nc.sbuf_tensor("proj_sb", [D, SH], F16))
        a8_sb = ctx.enter_context(nc.sbuf_tensor("a8_sb", [128, 2, KT, HW], F8))
        junk = ctx.enter_context(nc.sbuf_tensor("junk", [128, 640], F8))
        out_sb = ctx.enter_context(nc.sbuf_tensor("out_sb", [D, SH], F16))
        po2 = ctx.enter_context(nc.psum_tensor("po2", [D, SH], F32))
        scr = ctx.enter_context(nc.psum_tensor("scr", [128, 512], F32))

        # one semaphore PER chunk/piece: a single cumulative counter is
        # UNSOUND — the 16 SDMA engines complete their slices of queued
        # DMAs independently, so fast engines running ahead on chunk c+1
        # can raise the total to 16*(c+1) while a lagging engine (7/15
        # lag ~1 us) hasn't landed chunk c; the matmul then reads
        # unlanded SBUF (fp8 0xFF garbage = NaN). Observed ~1/6 runs.
        a_sems = [ctx.enter_context(nc.semaphore(f"a_sem{c}"))
                  for c in range(len(CHUNKS))]
        phl_sems = [ctx.enter_context(nc.semaphore(f"phl_sem{i}"))
                    for i in range(len(PHL_PC))]
        proj_sem = ctx.enter_context(nc.semaphore("proj_sem"))
        pe_h = [ctx.enter_context(nc.semaphore(f"pe_h{h}")) for h in range(2)]
        dve_o = [ctx.enter_context(nc.semaphore(f"dve_o{h}")) for h in range(2)]
        dma_o = ctx.enter_context(nc.semaphore("dma_o"))
        block = ctx.enter_context(nc.Block(no_gpsimd_drain=True))

        @block.sync
        def _(sync):
            # the whole A stream, in consumption order, on HWDGE ring 1
            off = {0: 0, 1: KT * HW}
            cs = {0: 0, 1: 0}
            for ci, (h, w) in enumerate(CHUNKS):
                c0 = cs[h]
                sync.dma_start(
                    a8_sb[:, h, c0:c0 + w, :],
                    a8[:, off[h] + c0 * HW:off[h] + (c0 + w) * HW],
                ).then_inc(a_sems[ci], 16)
                cs[h] = c0 + w
            # no final wait: the NEFF epilogue drains the DMA queues.

        @block.scalar
        def _(scalar):
            # phl + proj early on HWDGE ring 2 (concurrent with ring 1).
            # Deferring phl piece 2 + proj behind chunk 3's receipt was
            # measured WORSE (interleaved A/B: 42.9 vs 41.2 us mean) —
            # keep everything up front.
            for i, w in enumerate(PHL_PC):
                scalar.dma_start(
                    phl_sb[:, PHL_CS[i]:PHL_CS[i] + w, :],
                    phl[:, PHL_CS[i] * D:(PHL_CS[i] + w) * D],
                ).then_inc(phl_sems[i], 16)
            scalar.dma_start(proj_sb[:], proj[:]).then_inc(proj_sem, 16)
            # output halves as soon as each combine lands; walrus codegen
            # requires a completion inc on every DMA, but nothing WAITS on
            # dma_o — the NEFF epilogue's dma_reset drains the queue.
            for h in range(2):
                hsl = slice(h * HW, (h + 1) * HW)
                scalar.wait_ge(dve_o[h], 1)
                scalar.dma_start(outT[:, hsl], out_sb[:, hsl]).then_inc(
                    dma_o, 16)

        @block.tensor
        def _(tensor):
            # HAM warm-up on garbage SBUF (never read; fp8 NaNs are inert)
            for _ in range(N_JUNK):
                tensor.matmul(scr[:, 0:512], junk[:, 0:128], junk[:, 128:640],
                              start=True, stop=True)
            chunks_done = 0
            qt = 0
            for h in range(2):
                hsl = slice(h * HW, (h + 1) * HW)
                covered = -1
                for ci, w in enumerate(CHUNKS_H0 if h == 0 else CHUNKS_H1):
                    tensor.wait_ge(a_sems[chunks_done], 16)
                    chunks_done += 1
                    c0 = sum((CHUNKS_H0 if h == 0 else CHUNKS_H1)[:ci])
                    if h == 0:
                        while covered < c0 + w - 1:
                            tensor.wait_ge(phl_sems[qt], 16)
                            covered = PHL_CS[qt] + PHL_PC[qt] - 1
                            qt += 1
                    for t in range(w // 2):
                        k = c0 + 2 * t
                        last = k == KT - 2
                        mm = tensor.matmul(
                            po2[:, hsl],
                            phl_sb[:, k:k + 2, :],
                            a8_sb[:, h, k:k + 2, :],
                            start=k == 0,
                            stop=last,
                            perf_mode=mybir.MatmulPerfMode.DoubleRow,
                        )
                        if last:
                            mm.then_inc(pe_h[h], 1)
                    if h == 0:
                        for _ in range(GAP_JUNK.get(ci, 0)):
                            tensor.matmul(scr[:, 0:256], junk[:, 0:128],
                                          junk[:, 128:384],
                                          start=True, stop=True)

        @block.vector
        def _(vector):
            vector.wait_ge(proj_sem, 16)
            for h in range(2):
                hsl = slice(h * HW, (h + 1) * HW)
                vector.wait_ge(pe_h[h], 1)
                vector.tensor_add(
                    out_sb[:, hsl], po2[:, hsl], proj_sb[:, hsl]
                ).then_inc(dve_o[h], 1)

    nc.compile()
    return nc


def _prep_in_maps(object_features, relationship_features, adjacency_matrix,
                  W_obj, b_obj, W_nobj, b_nobj, W_rel, b_rel,
                  W_skip, b_skip):
    x = np.ascontiguousarray(object_features, dtype=np.float32)
    r = np.ascontiguousarray(relationship_features, dtype=np.float32)
    A = np.asarray(adjacency_matrix, dtype=np.float32)

    # P = x @ W_nobj, e4m3, k-tiled: phl[p, k*64+f] = P[k*128+p, f]
    P = x @ np.asarray(W_nobj, dtype=np.float32)                 # [N, D]
    phi = P.astype(ml_dtypes.float8_e4m3)
    phl = np.ascontiguousarray(
        phi.reshape(KT, 128, D).transpose(1, 0, 2).reshape(128, KT * D))

    # proj = x @ (W_obj+W_skip) + r @ W_rel + biases + colsum(A) x b_nobj
    colsum = A.sum(axis=0, dtype=np.float32)                     # [N]
    proj_full = (
        x @ (np.asarray(W_obj) + np.asarray(W_skip))
        + r @ np.asarray(W_rel)
        + (np.asarray(b_obj) + np.asarray(b_rel) + np.asarray(b_skip))[None, :]
        + colsum[:, None] * np.asarray(b_nobj)[None, :]
    ).T.astype(np.float16)                                       # [D, N]

    in_maps = []
    for m in range(M):
        sl = slice(m * SH, (m + 1) * SH)
        # column-major pre-tile: [k, p, h, j] -> [p, h, k, j]; exact fp8
        blk = A[:, sl].astype(ml_dtypes.float8_e4m3)             # [8192, 1024]
        blk = np.ascontiguousarray(
            blk.reshape(KT, 128, 2, HW).transpose(1, 2, 0, 3)
            .reshape(128, 2 * KT * HW))
        in_maps.append({
            "phl": phl,
            "proj": np.ascontiguousarray(proj_full[:, sl]),
            "a8": blk,
        })
    return in_maps


def run(inputs: dict, **run_kwargs):
    """Build (cached), run on cores 0-7, return (output, BassKernelResults)."""
    if "nc" not in _BUILT:
        _BUILT["nc"] = build_bass()
    nc = _BUILT["nc"]
    in_maps = _prep_in_maps(
        inputs["object_features"], inputs["relationship_features"],
        inputs["adjacency_matrix"],
        inputs["W_obj"], inputs["b_obj"], inputs["W_nobj"], inputs["b_nobj"],
        inputs["W_rel"], inputs["b_rel"], inputs["W_skip"], inputs["b_skip"],
    )
    last_err = None
    for attempt in range(3):
        try:
            res = bass_utils.run_bass_kernel_spmd(
                nc, in_maps, core_ids=list(range(M)), **run_kwargs
            )
            break
        except Exception as e:  # transient NRT device errors do occur
            last_err = e
            if attempt == 2:
                raise
            import time
            time.sleep(2.0)
    out = np.concatenate(
        [res.results[m]["outT"].T for m in range(M)], axis=0
    ).astype(np.float32)
    return out, res


def kernel(**inputs) -> np.ndarray:
    out, _ = run(inputs)
    return out
